# revision 14
# baseline (speedup 1.0000x reference)
"""MentionScoresHead Trainium2 kernel.

Computes the reference:
    logits = bert_output @ W.T + b                      (B, L, 3)
    start/end/mention = masked logit channels
    scores[b, s, e] = start[s] + end[e] + cumsum(mention)[s..e]
    output: scores gathered at static (s, e) pairs with e - s + 1 <= 10,
    plus static bounds.

Only a thin diagonal band (e in [s, s+9]) of the (L, L) score matrix is
ever finite; everything else is exactly -inf.  The device computes the
band (B, L, 10); the host scatters it into the -inf-filled static layout
(pure static index arithmetic, no compute).

Sharding: data-parallel over batch, 2 batches per core on 8 cores.

Device algorithm per batch:
  1. x is pre-masked (x * mask) host-side, then split into fp16 hi/lo
     pairs (x = xh + xl captures 21 mantissa bits; the fp32-PSUM matmul
     (wh+wl)@(xh+xl) is as accurate as a plain fp32 matmul, ~1e-6 rel).
     The fp16 halves are loaded with hardware DMA-transpose (2-byte
     dtype) directly into x^T layout [d, l] -- no PE transposes and no
     PSUM->SBUF round trip.
  2. PE matmul with a packed weight lhsT [128, 35] (cols 0-2 = Wh^T,
     cols 32-34 = Wl^T) accumulating all four split products into one
     PSUM tile [35, L]: logits^T = rows 0-2 + rows 32-34.
  3. ACT moves rows 0-2 to SBUF adding bias b; DVE adds rows 32-34.
  4. DVE prefix-scan of the mention row -> incl cumsum P (P[0]=0).
  5. Tiny SBUF->SBUF "deal" DMAs redistribute the 1-partition rows into
     (128, w) tiles with s = 8p + t:  Pd[p,j] = P[8p+j],
     Ed[p,j] = end[8p+j], Sd[p,t] = start[8p+t].
  6. band[s=8p+t, d] = (end[e] + minf[e] + incl[e]) + start[s] - excl[s]
     with e = s + d, computed as wide DVE ops; minf (0 / -inf, host
     precomputed in dealt layout) injects the exact -inf at masked e.
"""

import numpy as np

B, L, D = 16, 1024, 768
NCORES = 8
BPC = B // NCORES  # batches per core
BANDW = 10  # MAX_MENTION_LEN
NDB = D // 128  # 6 d-blocks


# ----------------------------------------------------------------------------
# Static output layout (pure function of L and MAX_MENTION_LEN).
# ----------------------------------------------------------------------------
def _static_layout():
    s = np.arange(L)
    rowlen = np.minimum(L, s + BANDW)  # entries e in [0, min(L-1, s+9)]
    offset = np.zeros(L + 1, np.int64)
    offset[1:] = np.cumsum(rowlen)
    K = int(offset[-1])
    nband = np.minimum(BANDW, L - s)  # finite-band entries per row
    s_rep = np.repeat(s, nband)
    d_rep = np.concatenate([np.arange(n) for n in nband])
    flatpos = (offset[s_rep] + s_rep + d_rep).astype(np.int64)  # into (K,)
    band_idx = (s_rep * BANDW + d_rep).astype(np.int64)  # into (L*10,)
    return K, flatpos, band_idx


def _static_bounds():
    # identical to reference._static_bounds_and_keep
    s = np.arange(L)[:, None]
    e = np.arange(L)[None, :]
    keep = ((e - s + 1) <= BANDW).reshape(-1)
    bounds = np.stack(
        [np.broadcast_to(s, (L, L)), np.broadcast_to(e, (L, L))], axis=-1
    ).reshape(-1, 2)
    return bounds[keep]


K, FLATPOS, BAND_IDX = _static_layout()
BOUNDS = _static_bounds()

_DEAL_IDX = (8 * np.arange(128))[:, None] + np.arange(18)[None, :]  # (128, 18)

_COMPILED = None


def _build_bass():
    import concourse.bass as bass
    import concourse.bacc as bacc
    import concourse.mybir as mybir
    import concourse.tile as tile
    from contextlib import ExitStack

    f32 = mybir.dt.float32
    f16 = mybir.dt.float16
    ADD = mybir.AluOpType.add
    SUB = mybir.AluOpType.subtract
    BYPASS = mybir.AluOpType.bypass

    nc = bacc.Bacc("TRN2", target_bir_lowering=False, debug=False)
    # inputs: fp16 hi/lo halves of masked x, HOST-pre-transposed to x^T
    # layout: xsplit[b, j, dd, hl*L + l] = split(x)[hl][b, l, 128j+dd]
    xsplit = nc.declare_dram_parameter(
        "xsplit", [BPC, NDB, 128, 2 * L], f16, isOutput=False
    )
    wtp = nc.declare_dram_parameter("wtp", [NDB, 128, 35], f16, isOutput=False)
    minfd = nc.declare_dram_parameter("minfd", [BPC, 128, 18], f32, isOutput=False)
    biasc = nc.declare_dram_parameter("biasc", [3, 1], f32, isOutput=False)
    band = nc.declare_dram_parameter("band", [BPC, L, BANDW], f32, isOutput=True)

    with tile.TileContext(nc) as tc, ExitStack() as ctx:
        consts = ctx.enter_context(tc.tile_pool(name="consts", bufs=1))
        xtpool = ctx.enter_context(tc.tile_pool(name="xt", bufs=12))
        psL = ctx.enter_context(tc.tile_pool(name="psL", bufs=2, space="PSUM"))
        rows = ctx.enter_context(tc.tile_pool(name="rows", bufs=2))
        small = ctx.enter_context(tc.tile_pool(name="small", bufs=2))

        wt_sb = consts.tile([128, NDB, 35], f16)
        nc.sync.dma_start(wt_sb[:, :, :], wtp.rearrange("j p k -> p j k"))
        bias_sb = consts.tile([3, 1], f32)
        nc.scalar.dma_start(bias_sb[:, :], biasc[:, :])

        for b in range(BPC):
            minf_sb = small.tile([128, 18], f32, tag="minf")
            nc.scalar.dma_start(minf_sb[:, :], minfd[b])

            # chunked linear loads of the host-pre-transposed fp16 halves,
            # one per d-block so matmuls start as soon as chunk 0 lands:
            # xc[j][p, hl*L + l] = x_hl[b, l, 128j + p]
            eng = nc.sync if b % 2 == 0 else nc.scalar
            xc = []
            for j in range(NDB):
                t = xtpool.tile([128, 2 * L], f16, tag="xt")
                eng.dma_start(t[:, :], xsplit[b, j])
                xc.append(t)

            # logits^T accumulation: rows 0-2 = Wh^T @ x_hl, rows 32-34 = Wl^T
            # j-outer so each chunk is consumed as it arrives
            ps = psL.tile([35, L], f32)
            for j in range(NDB):
                for h in range(2):
                    for hl in range(2):
                        nc.tensor.matmul(
                            ps[:, 512 * h : 512 * (h + 1)],
                            wt_sb[:, j, :],
                            xc[j][:, hl * L + 512 * h : hl * L + 512 * (h + 1)],
                            start=(j == 0 and hl == 0),
                            stop=(j == NDB - 1 and hl == 1),
                        )

            # logits^T -> SBUF: hi rows + bias via ACT, then += lo rows (DVE)
            lsb0 = rows.tile([3, 1040], f32, tag="lsb0")
            nc.scalar.activation(
                lsb0[:, 0:1024],
                ps[0:3, :],
                mybir.ActivationFunctionType.Identity,
                bias=bias_sb[:, 0:1],
                scale=1.0,
            )
            lsb = rows.tile([3, 1040], f32, tag="lsb")
            nc.vector.memset(lsb[:, 1024:1040], 0.0)
            nc.vector.tensor_add(lsb[:, 0:1024], lsb0[:, 0:1024], ps[32:35, :])

            # inclusive cumsum of mention row (row 0) -> P[1:1025]; P[0] = 0
            P = rows.tile([1, 1056], f32, tag="pscan")
            nc.vector.memset(P[0:1, 0:1], 0.0)
            nc.vector.memset(P[0:1, 1025:1056], 0.0)
            nc.vector.tensor_tensor_scan(
                P[0:1, 1:1025],
                lsb[0:1, 0:1024],
                lsb[0:1, 0:1024],
                initial=0.0,
                op0=ADD,
                op1=BYPASS,
            )

            # deal DMAs: redistribute rows into (128, w), s = 8p + t
            lfull = lsb[:, :]
            lpitch = int(lfull.ap[0][0])
            pfull = P[:, :]
            ppitch = int(pfull.ap[0][0])

            Pd = small.tile([128, 18], f32, tag="pd")
            nc.sync.dma_start(
                Pd[:, :],
                bass.AP(pfull.tensor, pfull.offset, [[ppitch, 1], [8, 128], [1, 18]]),
            )
            Sd = small.tile([128, 8], f32, tag="sd")
            nc.scalar.dma_start(
                Sd[:, :],
                bass.AP(
                    lfull.tensor,
                    lfull.offset + lpitch,
                    [[lpitch, 1], [8, 128], [1, 8]],
                ),
            )
            Ed = small.tile([128, 18], f32, tag="ed")
            nc.sync.dma_start(
                Ed[:, :],
                bass.AP(
                    lfull.tensor,
                    lfull.offset + 2 * lpitch,
                    [[lpitch, 1], [8, 128], [1, 18]],
                ),
            )

            # C[p, j] = end[8p+j] + minf[8p+j] + incl[8p+j]
            C1 = small.tile([128, 18], f32, tag="c1")
            nc.vector.tensor_add(C1[:, 0:17], Ed[:, 0:17], minf_sb[:, 0:17])
            C = small.tile([128, 18], f32, tag="c")
            nc.vector.tensor_add(C[:, 0:17], C1[:, 0:17], Pd[:, 1:18])

            # band[s=8p+t, d] = C[p, t+d] + start[s] - excl[s]
            bandt = small.tile([128, 80], f32, tag="band")
            for t in range(8):
                nc.vector.tensor_scalar(
                    bandt[:, BANDW * t : BANDW * (t + 1)],
                    C[:, t : t + BANDW],
                    Sd[:, t : t + 1],
                    Pd[:, t : t + 1],
                    ADD,
                    SUB,
                )
            nc.scalar.dma_start(
                band[b].rearrange("(p t) d -> p (t d)", p=128), bandt[:, :]
            )

    nc.compile()
    return nc


def _get_compiled():
    global _COMPILED
    if _COMPILED is None:
        _COMPILED = _build_bass()
    return _COMPILED


_CH_PERM = [2, 0, 1]  # device channel order: mention, start, end


def _host_prep(bert_output, input_mask, W, b):
    mask_f = input_mask.astype(np.float32)
    xm = bert_output * mask_f[:, :, None]
    xh = xm.astype(np.float16)
    xl = (xm - xh.astype(np.float32)).astype(np.float16)
    # (B, L, D) -> (B, NDB, 2, L, 128)
    xs = np.stack([xh, xl], axis=1)  # (B, 2, L, D)
    # -> (B, NDB, 128, 2, L): xsplit[b, j, dd, hl, l] = x_hl[b, l, 128j+dd]
    xs = xs.reshape(B, 2, L, NDB, 128).transpose(0, 3, 4, 1, 2)
    xsplit = np.ascontiguousarray(xs).reshape(B, NDB, 128, 2 * L)

    Wp = np.asarray(W, np.float32)[_CH_PERM]  # (3, D) mention, start, end
    wh = Wp.astype(np.float16)
    wl = (Wp - wh.astype(np.float32)).astype(np.float16)
    wtp = np.zeros((NDB, 128, 35), np.float16)
    for j in range(NDB):
        wtp[j, :, 0:3] = wh[:, 128 * j : 128 * (j + 1)].T
        wtp[j, :, 32:35] = wl[:, 128 * j : 128 * (j + 1)].T

    biasc = np.ascontiguousarray(
        np.asarray(b, np.float32)[_CH_PERM].reshape(3, 1)
    )
    minf_pad = np.full((B, 1040), -np.inf, np.float32)
    minf_pad[:, :1024] = np.where(input_mask, np.float32(0.0), np.float32(-np.inf))
    minfd = np.ascontiguousarray(minf_pad[:, _DEAL_IDX])  # (B, 128, 18)
    return xsplit, wtp, biasc, minfd


def run_device(bert_output, input_mask, W, b, trace=False, **kw):
    from concourse.bass_utils import run_bass_kernel_spmd

    nc = _get_compiled()
    xsplit, wtp, biasc, minfd = _host_prep(bert_output, input_mask, W, b)
    in_maps = [
        {
            "xsplit": xsplit[BPC * c : BPC * (c + 1)],
            "wtp": wtp,
            "minfd": minfd[BPC * c : BPC * (c + 1)],
            "biasc": biasc,
        }
        for c in range(NCORES)
    ]
    out = run_bass_kernel_spmd(nc, in_maps, list(range(NCORES)), trace=trace, **kw)
    bandf = np.concatenate([r["band"] for r in out.results], axis=0)  # (B, L, 10)
    return bandf, out


def assemble(bandf):
    scores = np.full((B, K), -np.inf, np.float32)
    scores[:, FLATPOS] = bandf.reshape(B, -1)[:, BAND_IDX]
    bounds = np.broadcast_to(BOUNDS.astype(np.int32)[None], (B, K, 2))
    return scores, bounds


def kernel(bert_output, input_mask, W, b):
    bert_output = np.asarray(bert_output, np.float32)
    input_mask = np.asarray(input_mask, bool)
    W = np.asarray(W, np.float32)
    b = np.asarray(b, np.float32)
    bandf, _ = run_device(bert_output, input_mask, W, b)
    return assemble(bandf)


# revision 17
# speedup vs baseline: 1.1891x; 1.1891x over previous
"""MentionScoresHead Trainium2 kernel.

Computes the reference:
    logits = bert_output @ W.T + b                      (B, L, 3)
    start/end/mention = masked logit channels
    scores[b, s, e] = start[s] + end[e] + cumsum(mention)[s..e]
    output: scores gathered at static (s, e) pairs with e - s + 1 <= 10,
    plus static bounds.

Only a thin diagonal band (e in [s, s+9]) of the (L, L) score matrix is
ever finite; everything else is exactly -inf.  The device computes the
band (B, L, 10); the host scatters it into the -inf-filled static layout
(pure static index arithmetic, no compute).

Sharding: data-parallel over batch, 2 batches per core on 8 cores.

Device algorithm per batch:
  1. x is pre-masked (x * mask) host-side, then split into fp16 hi/lo
     pairs (x = xh + xl captures 21 mantissa bits; the fp32-PSUM matmul
     (wh+wl)@(xh+xl) is as accurate as a plain fp32 matmul, ~1e-6 rel).
     The fp16 halves are loaded with hardware DMA-transpose (2-byte
     dtype) directly into x^T layout [d, l] -- no PE transposes and no
     PSUM->SBUF round trip.
  2. PE matmul with a packed weight lhsT [128, 35] (cols 0-2 = Wh^T,
     cols 32-34 = Wl^T) accumulating all four split products into one
     PSUM tile [35, L]: logits^T = rows 0-2 + rows 32-34.
  3. ACT moves rows 0-2 to SBUF adding bias b; DVE adds rows 32-34.
  4. DVE prefix-scan of the mention row -> incl cumsum P (P[0]=0).
  5. Tiny SBUF->SBUF "deal" DMAs redistribute the 1-partition rows into
     (128, w) tiles with s = 8p + t:  Pd[p,j] = P[8p+j],
     Ed[p,j] = end[8p+j], Sd[p,t] = start[8p+t].
  6. band[s=8p+t, d] = (end[e] + minf[e] + incl[e]) + start[s] - excl[s]
     with e = s + d, computed as wide DVE ops; minf (0 / -inf, host
     precomputed in dealt layout) injects the exact -inf at masked e.
"""

import numpy as np

B, L, D = 16, 1024, 768
NCORES = 8
BPC = B // NCORES  # batches per core
BANDW = 10  # MAX_MENTION_LEN
NDB = D // 128  # 6 d-blocks


# ----------------------------------------------------------------------------
# Static output layout (pure function of L and MAX_MENTION_LEN).
# ----------------------------------------------------------------------------
def _static_layout():
    s = np.arange(L)
    rowlen = np.minimum(L, s + BANDW)  # entries e in [0, min(L-1, s+9)]
    offset = np.zeros(L + 1, np.int64)
    offset[1:] = np.cumsum(rowlen)
    K = int(offset[-1])
    nband = np.minimum(BANDW, L - s)  # finite-band entries per row
    s_rep = np.repeat(s, nband)
    d_rep = np.concatenate([np.arange(n) for n in nband])
    flatpos = (offset[s_rep] + s_rep + d_rep).astype(np.int64)  # into (K,)
    band_idx = (s_rep * BANDW + d_rep).astype(np.int64)  # into (L*10,)
    return K, flatpos, band_idx


def _static_bounds():
    # identical to reference._static_bounds_and_keep
    s = np.arange(L)[:, None]
    e = np.arange(L)[None, :]
    keep = ((e - s + 1) <= BANDW).reshape(-1)
    bounds = np.stack(
        [np.broadcast_to(s, (L, L)), np.broadcast_to(e, (L, L))], axis=-1
    ).reshape(-1, 2)
    return bounds[keep]


K, FLATPOS, BAND_IDX = _static_layout()
BOUNDS = _static_bounds()

_DEAL_IDX = (8 * np.arange(128))[:, None] + np.arange(18)[None, :]  # (128, 18)

_COMPILED = None


def _build_bass():
    import concourse.bass as bass
    import concourse.bacc as bacc
    import concourse.mybir as mybir
    import concourse.tile as tile
    from contextlib import ExitStack

    f32 = mybir.dt.float32
    f16 = mybir.dt.float16
    ADD = mybir.AluOpType.add
    SUB = mybir.AluOpType.subtract
    BYPASS = mybir.AluOpType.bypass

    nc = bacc.Bacc("TRN2", target_bir_lowering=False, debug=False)
    # inputs: fp16 hi/lo halves of masked x, HOST-pre-transposed to x^T
    # layout: xsplit[b, j, dd, hl*L + l] = split(x)[hl][b, l, 128j+dd]
    xsplit = nc.declare_dram_parameter(
        "xsplit", [BPC, NDB, 128, 2 * L], f16, isOutput=False
    )
    wtp = nc.declare_dram_parameter("wtp", [NDB, 128, 35], f16, isOutput=False)
    minfd = nc.declare_dram_parameter("minfd", [BPC, 128, 18], f32, isOutput=False)
    biasc = nc.declare_dram_parameter("biasc", [3, 1], f32, isOutput=False)
    band = nc.declare_dram_parameter("band", [BPC, L, BANDW], f32, isOutput=True)

    with tile.TileContext(nc) as tc, ExitStack() as ctx:
        consts = ctx.enter_context(tc.tile_pool(name="consts", bufs=1))
        xtpool = ctx.enter_context(tc.tile_pool(name="xt", bufs=12))
        psL = ctx.enter_context(tc.tile_pool(name="psL", bufs=2, space="PSUM"))
        psD = ctx.enter_context(tc.tile_pool(name="psD", bufs=1, space="PSUM"))
        rows = ctx.enter_context(tc.tile_pool(name="rows", bufs=2))
        small = ctx.enter_context(tc.tile_pool(name="small", bufs=2))

        wt_sb = consts.tile([128, NDB, 35], f16)
        nc.sync.dma_start(wt_sb[:, :, :], wtp.rearrange("j p k -> p j k"))
        bias_sb = consts.tile([3, 1], f32)
        nc.scalar.dma_start(bias_sb[:, :], biasc[:, :])

        # PE warm-up: dummy matmuls during the load phase flip HAM to 8/8
        # before the real matmuls issue (transposes/none here, but cold-start
        # otherwise costs ~2x on every real matmul).
        dum = consts.tile([128, 512], f16)
        nc.vector.memset(dum[:, :], 0.0)
        dps = psD.tile([35, 512], f32)
        for w in range(14):
            nc.tensor.matmul(
                dps[:, :], wt_sb[:, 0, :], dum[:, :], start=(w == 0), stop=(w == 13)
            )

        for b in range(BPC):
            minf_sb = small.tile([128, 18], f32, tag="minf")
            nc.scalar.dma_start(minf_sb[:, :], minfd[b])

            # chunked linear loads of the host-pre-transposed fp16 halves,
            # one per d-block so matmuls start as soon as chunk 0 lands:
            # xc[j][p, hl*L + l] = x_hl[b, l, 128j + p]
            eng = nc.sync if b % 2 == 0 else nc.scalar
            xc = []
            for j in range(NDB):
                t = xtpool.tile([128, 2 * L], f16, tag="xt")
                eng.dma_start(t[:, :], xsplit[b, j])
                xc.append(t)

            # logits^T accumulation: rows 0-2 = Wh^T @ x_hl, rows 32-34 = Wl^T
            # h-outer so the h=0 half-group completes as soon as the last
            # chunk lands and the tail can start during the h=1 group.
            ps = psL.tile([35, L], f32)
            lsb0 = rows.tile([3, 1040], f32, tag="lsb0")
            lsb = rows.tile([3, 1040], f32, tag="lsb")
            nc.vector.memset(lsb[:, 1024:1040], 0.0)
            P = rows.tile([1, 1056], f32, tag="pscan")
            nc.vector.memset(P[0:1, 0:1], 0.0)
            nc.vector.memset(P[0:1, 1025:1056], 0.0)

            for h in range(2):
                n_mm = 0
                for j in range(NDB):
                    for hl in range(2):
                        nc.tensor.matmul(
                            ps[:, 512 * h : 512 * (h + 1)],
                            wt_sb[:, j, :],
                            xc[j][:, hl * L + 512 * h : hl * L + 512 * (h + 1)],
                            start=(n_mm == 0),
                            stop=(n_mm == 2 * NDB - 1),
                        )
                        n_mm += 1
                # per-half tail: hi rows + bias via ACT, += lo rows, scan chunk
                hs = slice(512 * h, 512 * (h + 1))
                nc.scalar.activation(
                    lsb0[:, hs],
                    ps[0:3, hs],
                    mybir.ActivationFunctionType.Identity,
                    bias=bias_sb[:, 0:1],
                    scale=1.0,
                )
                nc.vector.tensor_add(lsb[:, hs], lsb0[:, hs], ps[32:35, hs])
                nc.vector.tensor_tensor_scan(
                    P[0:1, 1 + 512 * h : 1 + 512 * (h + 1)],
                    lsb[0:1, hs],
                    lsb[0:1, hs],
                    initial=(0.0 if h == 0 else P[0:1, 512:513]),
                    op0=ADD,
                    op1=BYPASS,
                )

            # deal DMAs: redistribute rows into (128, w), s = 8p + t
            lfull = lsb[:, :]
            lpitch = int(lfull.ap[0][0])
            pfull = P[:, :]
            ppitch = int(pfull.ap[0][0])

            Pd = small.tile([128, 18], f32, tag="pd")
            nc.sync.dma_start(
                Pd[:, :],
                bass.AP(pfull.tensor, pfull.offset, [[ppitch, 1], [8, 128], [1, 18]]),
            )
            Sd = small.tile([128, 8], f32, tag="sd")
            nc.scalar.dma_start(
                Sd[:, :],
                bass.AP(
                    lfull.tensor,
                    lfull.offset + lpitch,
                    [[lpitch, 1], [8, 128], [1, 8]],
                ),
            )
            Ed = small.tile([128, 18], f32, tag="ed")
            nc.sync.dma_start(
                Ed[:, :],
                bass.AP(
                    lfull.tensor,
                    lfull.offset + 2 * lpitch,
                    [[lpitch, 1], [8, 128], [1, 18]],
                ),
            )

            # C[p, j] = end[8p+j] + minf[8p+j] + incl[8p+j]
            C1 = small.tile([128, 18], f32, tag="c1")
            nc.vector.tensor_add(C1[:, 0:17], Ed[:, 0:17], minf_sb[:, 0:17])
            C = small.tile([128, 18], f32, tag="c")
            nc.vector.tensor_add(C[:, 0:17], C1[:, 0:17], Pd[:, 1:18])
            # A[p, t] = start[8p+t] - excl[8p+t]
            A = small.tile([128, 8], f32, tag="a")
            nc.vector.tensor_tensor(A[:, :], Sd[:, :], Pd[:, 0:8], SUB)

            # band[s=8p+t, d] = C[p, t+d] + A[p, t] in two wide DVE ops
            af = A[:, :]
            apitch = int(af.ap[0][0])
            cf = C[:, :]
            cpitch = int(cf.ap[0][0])
            tmp = small.tile([128, 80], f32, tag="tmp")
            nc.vector.tensor_copy(
                tmp[:, :].rearrange("p (t d) -> p t d", t=8),
                bass.AP(af.tensor, af.offset, [[apitch, 128], [1, 8], [0, 10]]),
            )
            bandt = small.tile([128, 80], f32, tag="band")
            nc.vector.tensor_tensor(
                bandt[:, :].rearrange("p (t d) -> p t d", t=8),
                bass.AP(cf.tensor, cf.offset, [[cpitch, 128], [1, 8], [1, 10]]),
                tmp[:, :].rearrange("p (t d) -> p t d", t=8),
                ADD,
            )
            nc.scalar.dma_start(
                band[b].rearrange("(p t) d -> p (t d)", p=128), bandt[:, :]
            )

    nc.compile()
    return nc


def _get_compiled():
    global _COMPILED
    if _COMPILED is None:
        _COMPILED = _build_bass()
    return _COMPILED


_CH_PERM = [2, 0, 1]  # device channel order: mention, start, end


def _host_prep(bert_output, input_mask, W, b):
    mask_f = input_mask.astype(np.float32)
    xm = bert_output * mask_f[:, :, None]
    xh = xm.astype(np.float16)
    xl = (xm - xh.astype(np.float32)).astype(np.float16)
    # (B, L, D) -> (B, NDB, 2, L, 128)
    xs = np.stack([xh, xl], axis=1)  # (B, 2, L, D)
    # -> (B, NDB, 128, 2, L): xsplit[b, j, dd, hl, l] = x_hl[b, l, 128j+dd]
    xs = xs.reshape(B, 2, L, NDB, 128).transpose(0, 3, 4, 1, 2)
    xsplit = np.ascontiguousarray(xs).reshape(B, NDB, 128, 2 * L)

    Wp = np.asarray(W, np.float32)[_CH_PERM]  # (3, D) mention, start, end
    wh = Wp.astype(np.float16)
    wl = (Wp - wh.astype(np.float32)).astype(np.float16)
    wtp = np.zeros((NDB, 128, 35), np.float16)
    for j in range(NDB):
        wtp[j, :, 0:3] = wh[:, 128 * j : 128 * (j + 1)].T
        wtp[j, :, 32:35] = wl[:, 128 * j : 128 * (j + 1)].T

    biasc = np.ascontiguousarray(
        np.asarray(b, np.float32)[_CH_PERM].reshape(3, 1)
    )
    minf_pad = np.full((B, 1040), -np.inf, np.float32)
    minf_pad[:, :1024] = np.where(input_mask, np.float32(0.0), np.float32(-np.inf))
    minfd = np.ascontiguousarray(minf_pad[:, _DEAL_IDX])  # (B, 128, 18)
    return xsplit, wtp, biasc, minfd


def run_device(bert_output, input_mask, W, b, trace=False, **kw):
    from concourse.bass_utils import run_bass_kernel_spmd

    nc = _get_compiled()
    xsplit, wtp, biasc, minfd = _host_prep(bert_output, input_mask, W, b)
    in_maps = [
        {
            "xsplit": xsplit[BPC * c : BPC * (c + 1)],
            "wtp": wtp,
            "minfd": minfd[BPC * c : BPC * (c + 1)],
            "biasc": biasc,
        }
        for c in range(NCORES)
    ]
    out = run_bass_kernel_spmd(nc, in_maps, list(range(NCORES)), trace=trace, **kw)
    bandf = np.concatenate([r["band"] for r in out.results], axis=0)  # (B, L, 10)
    return bandf, out


def assemble(bandf):
    scores = np.full((B, K), -np.inf, np.float32)
    scores[:, FLATPOS] = bandf.reshape(B, -1)[:, BAND_IDX]
    bounds = np.broadcast_to(BOUNDS.astype(np.int32)[None], (B, K, 2))
    return scores, bounds


def kernel(bert_output, input_mask, W, b):
    bert_output = np.asarray(bert_output, np.float32)
    input_mask = np.asarray(input_mask, bool)
    W = np.asarray(W, np.float32)
    b = np.asarray(b, np.float32)
    bandf, _ = run_device(bert_output, input_mask, W, b)
    return assemble(bandf)
